# revision 59
# baseline (speedup 1.0000x reference)
"""Decode-step KV-cache attention kernel for 8 Trainium2 NeuronCores.

Tensor-parallel over heads (2 heads per core, all 32 batch rows on every
core); per-core differences live in host-sliced inputs.  All bulk data is
bf16 (tolerance is 2e-2; bf16 keeps norm-relative error ~4e-3 — fp8
anywhere in the attention path measures 2.4e-2+ because the attention
output is itself a noise-like average, so per-element quantization error
maps ~1:1 to relative output error).

Queue plan (the previous revision serialized the first K group behind a
3.1 MB W_in load on the same HWDGE queue and stalled every engine for
32 us):
  - scalar (HWDGE q10): K group tiles ONLY — first kt DMA issues at t=0.
  - gpsimd (SWDGE q0):  V group tiles ONLY.
  - sync   (HWDGE q1):  x, W_in (split q/k/v so the q columns land
    first and unblock the first scores), b_in, per-row v_new inserts,
    W_out (emitted late, it is only needed by phase 3), output stores.

Per-core pipeline (rows host-sorted by sequence length, descending;
adjacent small rows batched into shared 16 KB/partition DMA groups):
  1. QKV projection: x^T tiles (PE transposes), then q matmuls (winq),
     k_new matmuls (wink), v_new matmul (winv) — in that order so the
     first scores only wait on the 0.78 MB winq slice.
  2. Scores, per (row, head, tile): one matmul with the host-TRANSPOSED
     K tile [d, tokens] stationary and the q column moving -> scores
     [tokens, 1] in PSUM.  k_new/v_new are folded into the tiles at
     position L-1 beforehand.
  3. Exp per row over the packed score tiles (exact lengths, exact
     denominators).
  4. PV, per (row, tile): ONE matmul with the 2 probability columns as
     weights and the V tile [tokens, 257] streaming — column 256 of
     every V row-tile is hardwired to 1.0 host-side, so the softmax
     denominator accumulates in PSUM column 256 of the same bank and
     the separate denominator matmul (and its weight load) is gone.
  5. Per row: reciprocal, normalize out of PSUM, PE-transpose into
     [d, rowhead] layout.
  6. Out-project with bf16 W_out, PSUM->SBUF->HBM in 512-col chunks;
     host sums the 8 per-core partials and adds b_out.
"""

import math
import sys

import numpy as np
import ml_dtypes

sys.path.insert(0, "/opt/trn_rl_repo")

import concourse.bass as bass  # noqa: E402
import concourse.tile as tile  # noqa: E402
from concourse import bacc, mybir  # noqa: E402
from concourse.bass_utils import run_bass_kernel_spmd  # noqa: E402
from concourse.masks import make_identity  # noqa: E402

B, S_MAX, H, D = 32, 2048, 16, 128
E = H * D  # 2048
N_CORES = 8
H_LOC = H // N_CORES  # 2 heads per core
CLOC = H_LOC * D  # 256
VROW = CLOC + 1  # V tile row: 256 d-columns + 1 ones-column (denominator)
ET = E // 128  # 16 contraction tiles for the in-projection

F32 = mybir.dt.float32
BF16 = mybir.dt.bfloat16
NPBF = ml_dtypes.bfloat16
EXP = mybir.ActivationFunctionType.Exp

_build_cache: dict = {}
LAST_RESULT = None  # last BassKernelResults, for test harness introspection


def make_groups(nts):
    """Group adjacent rows so several small rows share one DMA + SBUF tile.
    Group size n at max-tile-count m keeps n*m <= 32 (16 KB/partition); a
    strict pad budget keeps the padding bytes negligible."""
    groups = []
    j = 0
    while j < B:
        m = nts[j]
        n = 1
        pad = 0
        while j + n < B and n < 16:
            m2 = max(m, nts[j + n])
            pad2 = pad + (m2 - m) * n + (m2 - nts[j + n])
            # cap at 31 tiles: keeps every DMA partition line <= 16 KB so no
            # transfer straddles the SDMA packet boundary
            if (n + 1) * m2 > 31 or pad2 > 2:
                break
            m, pad = m2, pad2
            n += 1
        groups.append(list(range(j, j + n)))
        j += n
    return groups


def _build(Ls: tuple, obs: tuple) -> bass.Bass:
    """Per-core Bass program. Ls = sorted (descending) seq lengths;
    obs[j] = original batch index of sorted row j."""
    nts = [(l + 127) // 128 for l in Ls]
    groups = make_groups(nts)
    g_of = {}
    r_of = {}
    k_rof = {}  # row's K base offset within its group tile (exact lengths)
    v_rof = {}  # row's V base offset within its group tile (exact tiles)
    for gi, grp in enumerate(groups):
        koff = 0
        voff = 0
        for r, j in enumerate(grp):
            g_of[j], r_of[j] = gi, r
            k_rof[j] = koff
            v_rof[j] = voff
            koff += 2 * Ls[j]
            voff += nts[j] * VROW
    k_goff = []  # element offsets of each group's K/V block in packed bufs
    v_goff = []
    k_gsz = []
    v_gsz = []
    ko = vo = 0
    for gi, grp in enumerate(groups):
        k_goff.append(ko)
        v_goff.append(vo)
        k_gsz.append(sum(2 * Ls[j] for j in grp))
        v_gsz.append(sum(nts[j] * VROW for j in grp))
        ko += 128 * k_gsz[gi]
        vo += 128 * v_gsz[gi]

    nc = bacc.Bacc("TRN2")
    x_d = nc.dram_tensor("x", [B, E], BF16, kind="ExternalInput")
    winkq_d = nc.dram_tensor(
        "winkq", [128, ET * 2 * CLOC], BF16, kind="ExternalInput"
    )
    winv_d = nc.dram_tensor("winv", [128, ET * CLOC], BF16, kind="ExternalInput")
    bin_d = nc.dram_tensor("bin", [1, 3 * CLOC], BF16, kind="ExternalInput")
    wout_d = nc.dram_tensor("wout", [128, H_LOC * E], BF16, kind="ExternalInput")
    kc_d = nc.dram_tensor("kc", [ko], BF16, kind="ExternalInput")
    vc_d = nc.dram_tensor("vc", [vo], BF16, kind="ExternalInput")
    out_d = nc.dram_tensor("out", [B, E], F32, kind="ExternalOutput")

    kc_base = kc_d[:]
    vc_base = vc_d[:]
    inv_sqrt_d = 1.0 / math.sqrt(D)

    with tile.TileContext(nc) as tc:
        with tc.tile_pool(name="const", bufs=1) as const:
            I64 = const.tile([64, 64], BF16)
            make_identity(nc, I64)
            I32 = I64[0:32, 0:32]
            ones_1x32 = const.tile([1, 32], BF16)
            nc.vector.memset(ones_1x32, 1.0)

            # All phase-1 weights go FIRST on the scalar queue — ahead of the
            # K groups in its FIFO — while the V stream is WAR-held behind
            # qT: during the first ~10 us the weight blob owns the SDMA
            # engines and lands at full rate.
            x_sb = const.tile([B, E], BF16)
            nc.scalar.dma_start(out=x_sb, in_=x_d[:])
            bin_sb = const.tile([1, 3 * CLOC], BF16)
            nc.scalar.dma_start(out=bin_sb, in_=bin_d[:])
            # k_new and q weight slices ride ONE transfer (one semaphore:
            # their consumers cannot be poisoned by each other's traffic).
            # It precedes the K groups in the scalar FIFO while the V stream
            # is WAR-held behind qT, so the weights land at full rate first.
            winkq_sb = const.tile([128, ET, 2, CLOC], BF16)
            nc.scalar.dma_start(out=winkq_sb, in_=winkq_d[:])
            winv_sb = const.tile([128, ET, CLOC], BF16)
            nc.scalar.dma_start(out=winv_sb, in_=winv_d[:])
            wout_sb = const.tile([128, H_LOC, E], BF16)
            # preload the exp table during the initial DMA window
            dummy_sb = const.tile([1, 2], F32)
            nc.scalar.activation(
                dummy_sb[0:1, 0:1], ones_1x32[0:1, 0:1], EXP, scale=1.0
            )

            xT_sb = const.tile([128, ET, B], BF16)
            v_new_sb = const.tile([B, CLOC], BF16)
            qT_sb = const.tile([128, H_LOC, B], BF16)
            k_newT_sb = const.tile([128, H_LOC, B], BF16)
            aT_sb = const.tile([128, H_LOC, 2 * B], BF16)
            out_sb = const.tile([B, E], F32)

            # ---------------- phase 1: fused QKV projection ----------------
            # q and k_new are produced directly in [d, row] orientation by
            # using the W_in columns as the output partition dim.  k_new
            # first (the per-row cache inserts consume it), then q, then
            # v_new from winv whose DMA is WAR-anchored behind qT so it can
            # never delay the score-critical winkq transfer.
            with tc.tile_pool(name="ph1ps", bufs=2, space="PSUM") as ph1ps:
                with tc.tile_pool(name="qkvps", bufs=1, space="PSUM") as qkvps:
                    for t in range(ET):
                        xt_ps = ph1ps.tile([128, B], BF16)
                        nc.tensor.transpose(
                            xt_ps, x_sb[:, t * 128 : (t + 1) * 128], I32
                        )
                        nc.vector.tensor_copy(xT_sb[:, t, :], xt_ps)
                    for sel, bbase, dst in (
                        (1, CLOC, k_newT_sb),
                        (0, 0, qT_sb),
                    ):
                        for h in range(H_LOC):
                            ps = qkvps.tile([128, B], F32, tag=f"qk{sel}{h}")
                            nc.tensor.matmul(
                                ps,
                                bin_sb[:, bbase + h * D : bbase + (h + 1) * D],
                                ones_1x32,
                                start=True,
                                stop=False,
                            )
                            for t in range(ET):
                                nc.tensor.matmul(
                                    ps,
                                    winkq_sb[
                                        :, t, sel, h * D : (h + 1) * D
                                    ],
                                    xT_sb[:, t, :],
                                    start=False,
                                    stop=(t == ET - 1),
                                )
                            nc.vector.tensor_copy(dst[:, h, :], ps)
                    v_ps = qkvps.tile([B, CLOC], F32, tag="v")
                    nc.tensor.matmul(
                        v_ps,
                        ones_1x32,
                        bin_sb[:, 2 * CLOC : 3 * CLOC],
                        start=True,
                        stop=False,
                    )
                    for t in range(ET):
                        nc.tensor.matmul(
                            v_ps,
                            xT_sb[:, t, :],
                            winv_sb[:, t, :],
                            start=False,
                            stop=(t == ET - 1),
                        )
                    nc.vector.tensor_copy(v_new_sb, v_ps)

            # ---------------- phase 2: scores -> exp -> PV ------------------
            with tc.tile_pool(name="scps", bufs=3, space="PSUM") as scps, \
                 tc.tile_pool(name="avps", bufs=2, space="PSUM") as avps, \
                 tc.tile_pool(name="tps", bufs=2, space="PSUM") as tps, \
                 tc.tile_pool(name="ktp", bufs=4) as ktp, \
                 tc.tile_pool(name="vtp", bufs=4) as vtp, \
                 tc.tile_pool(name="prp", bufs=4) as prp, \
                 tc.tile_pool(name="arp", bufs=3) as arp:
                kts = [None] * len(groups)
                vts = [None] * len(groups)
                prs = [None] * B
                GELEMS = 8192  # 16 KB/partition K group tiles
                VGELEMS = 32 * VROW  # V group tiles incl. ones-columns

                # The smallest rows (last 3 groups) are preloaded into
                # dedicated tiles on the otherwise-idle sync queue at t=0 and
                # processed mid-stream: the post-DMA drain then ends with the
                # 4th-last group instead of rows that only arrive at the very
                # end of the stream.
                S_groups = set(range(max(0, len(groups) - 3), len(groups)))
                for gi in sorted(S_groups):
                    kt = const.tile([128, k_gsz[gi]], BF16, tag=f"tk{gi}")
                    kts[gi] = kt
                    nc.sync.dma_start(
                        out=kt,
                        in_=bass.AP(
                            tensor=kc_base.tensor,
                            offset=k_goff[gi],
                            ap=[[k_gsz[gi], 128], [1, k_gsz[gi]]],
                        ),
                    )
                    vt = const.tile([128, v_gsz[gi]], BF16, tag=f"tv{gi}")
                    vts[gi] = vt
                    nc.sync.dma_start(
                        out=vt,
                        in_=bass.AP(
                            tensor=vc_base.tensor,
                            offset=v_goff[gi],
                            ap=[[v_gsz[gi], 128], [1, v_gsz[gi]]],
                        ),
                    )

                def anchor(dst_ap, src_ap):
                    """WAR anchor: a 1-element DVE write into the DMA target
                    region, reading a value produced by the stage the DMA
                    must trail — the transfer can then neither be hoisted
                    nor issued before that stage retires."""
                    nc.vector.tensor_copy(dst_ap, src_ap)

                emitted = []

                def anchor_src():
                    # prp rotates every 4 tiles, so this 6-back read aliases
                    # to the 2-back tile's buffer — an effective ~2-row trail,
                    # which measured best
                    if emitted:
                        return prs[emitted[-6] if len(emitted) >= 6
                                   else emitted[0]]
                    return None

                def emit_inserts(gi):
                    # cross-partition v_new inserts must go through DMA;
                    # hoisted to group level so their conservative
                    # cross-engine sem thresholds stay small
                    vt = vts[gi]
                    for jj in groups[gi]:
                        cj = Ls[jj] - 1
                        pn, tn = cj % 128, cj // 128
                        vjo = v_rof[jj]
                        oj = obs[jj]
                        nc.sync.dma_start(
                            out=vt[
                                pn : pn + 1,
                                vjo + tn * VROW : vjo + tn * VROW + CLOC,
                            ],
                            in_=v_new_sb[oj : oj + 1, :],
                        )

                def emit_scores(j):
                    L = Ls[j]
                    nt = nts[j]
                    gi, r = g_of[j], r_of[j]
                    ksz = k_gsz[gi]
                    if r == 0 and gi in S_groups:
                        emit_inserts(gi)
                    elif r == 0:
                        kt = ktp.tile([128, GELEMS], BF16)
                        kts[gi] = kt
                        src = anchor_src() if gi >= 1 else None
                        if src is not None:
                            anchor(kt[0:1, 0:1], src[0:1, 0, 0:1])
                        nc.scalar.dma_start(
                            out=kt[:, 0:ksz],
                            in_=bass.AP(
                                tensor=kc_base.tensor,
                                offset=k_goff[gi],
                                ap=[[ksz, 128], [1, ksz]],
                            ),
                        )
                        vt = vtp.tile([128, VGELEMS], BF16)
                        vts[gi] = vt
                        vsz = v_gsz[gi]
                        if gi < 2:
                            # V is not needed until the first PV (~LAG rows
                            # in); holding its head start keeps the startup
                            # window clear for weights + first K
                            anchor(vt[0:1, 0:1], qT_sb[0:1, 0, 0:1])
                        else:
                            src = anchor_src()
                            if src is not None:
                                anchor(vt[0:1, 0:1], src[0:1, 0, 0:1])
                        nc.gpsimd.dma_start(
                            out=vt[:, 0:vsz],
                            in_=bass.AP(
                                tensor=vc_base.tensor,
                                offset=v_goff[gi],
                                ap=[[vsz, 128], [1, vsz]],
                            ),
                        )
                        emit_inserts(gi)
                    kt = kts[gi]
                    ko = k_rof[j]  # this row's K base within the tile
                    # fold the new token in at column L-1
                    col = L - 1
                    ob = obs[j]
                    for h in range(H_LOC):
                        nc.vector.tensor_copy(
                            kt[:, ko + h * L + col : ko + h * L + col + 1],
                            k_newT_sb[:, h, ob : ob + 1],
                        )
                    sc = scps.tile([128, H_LOC, ET], F32)
                    pr = prp.tile([128, H_LOC, ET], BF16)
                    prs[j] = pr
                    rem = L - (nt - 1) * 128  # tokens in the last tile
                    for h in range(H_LOC):
                        for t in range(nt):
                            o = ko + h * L + t * 128
                            nc2 = 128 if t < nt - 1 else rem
                            nc.tensor.matmul(
                                sc[0:nc2, h, t : t + 1],
                                kt[:, o : o + nc2],
                                qT_sb[:, h, ob : ob + 1],
                                start=True,
                                stop=True,
                                skip_group_check=True,
                            )
                    if nt > 1:
                        nc.scalar.activation(
                            pr[:, :, 0 : nt - 1],
                            sc[:, :, 0 : nt - 1],
                            EXP,
                            scale=inv_sqrt_d,
                        )
                    nc.scalar.activation(
                        pr[0:rem, :, nt - 1 : nt],
                        sc[0:rem, :, nt - 1 : nt],
                        EXP,
                        scale=inv_sqrt_d,
                    )

                def emit_pv(j):
                    nt = nts[j]
                    gi, r = g_of[j], r_of[j]
                    vt = vts[gi]
                    vo = v_rof[j]
                    pr = prs[j]
                    # av columns 0:256 accumulate P@V; column 256 accumulates
                    # the softmax denominator via the ones-column of each V
                    # row-tile (same PSUM bank, same accumulation stream).
                    av = avps.tile([H_LOC, VROW], F32)
                    rem = Ls[j] - (nt - 1) * 128
                    for t in range(nt):
                        nc2 = 128 if t < nt - 1 else rem
                        nc.tensor.matmul(
                            av,
                            pr[0:nc2, :, t],
                            vt[0:nc2, vo + t * VROW : vo + (t + 1) * VROW],
                            start=(t == 0),
                            stop=(t == nt - 1),
                            skip_group_check=True,
                        )
                    ar = arp.tile([H_LOC, CLOC], BF16)
                    den2 = arp.tile([H_LOC, 2], F32, tag="dn")
                    nc.vector.reciprocal(den2[:, 1:2], av[:, CLOC : CLOC + 1])
                    nc.vector.tensor_scalar_mul(ar, av[:, 0:CLOC], den2[:, 1:2])
                    for h in range(H_LOC):
                        at_ps = tps.tile([128, H_LOC], BF16)
                        nc.tensor.transpose(
                            at_ps, ar[:, h * D : (h + 1) * D], I64[0:2, 0:2]
                        )
                        nc.vector.tensor_copy(
                            aT_sb[:, h, 2 * j : 2 * j + 2], at_ps
                        )

                LAG = 3
                WOUT_AT = 24  # start the 1 MB wout load ~75% through
                tail_rows = [
                    jj for gi in sorted(S_groups) for jj in groups[gi]
                ]
                head_rows = [
                    jj for jj in range(B) if g_of[jj] not in S_groups
                ]
                seq = head_rows[:6] + tail_rows + head_rows[6:]
                for i, j in enumerate(seq):
                    emit_scores(j)
                    emitted.append(j)
                    if i >= LAG:
                        emit_pv(seq[i - LAG])
                    if i == WOUT_AT:
                        # WAR anchor: the copy below reads the aT slot of the
                        # row finished LAG steps ago, so the wout DMA cannot
                        # be hoisted ahead of it — it streams only once ~75%
                        # of the pipeline has drained.
                        pv = seq[i - LAG]
                        nc.vector.tensor_copy(
                            wout_sb[0:1, 0, 0:2],
                            aT_sb[0:1, 0, 2 * pv : 2 * pv + 2],
                        )
                        nc.sync.dma_start(out=wout_sb, in_=wout_d[:])
                for i in range(B - LAG, B):
                    emit_pv(seq[i])

            # ---------------- phase 3: out-projection -----------------------
            with tc.tile_pool(name="outps", bufs=1, space="PSUM") as outps:
                    out_ps = outps.tile([B, E], F32)
                    # all matmuls first (independent PSUM banks accumulate
                    # back-to-back, keeping the PE ramped), copies + stores
                    # drain behind them
                    for j4 in range(4):
                        for h in range(H_LOC):
                            base = aT_sb[:, h, :]
                            lhsT = bass.AP(
                                tensor=base.tensor,
                                offset=base.offset + h,
                                ap=[base.ap[0], [2, B]],
                            )
                            nc.tensor.matmul(
                                out_ps[:, j4 * 512 : (j4 + 1) * 512],
                                lhsT,
                                wout_sb[:, h, j4 * 512 : (j4 + 1) * 512],
                                start=(h == 0),
                                stop=(h == H_LOC - 1),
                                skip_group_check=True,
                            )
                    for j4 in range(4):
                        s = slice(j4 * 512, (j4 + 1) * 512)
                        nc.vector.tensor_copy(out_sb[:, s], out_ps[:, s])
                        nc.sync.dma_start(out=out_d[:, s], in_=out_sb[:, s])
    nc.compile()
    return nc


def kernel(x, k_cache, v_cache, W_in, b_in, W_out, b_out, input_pos):
    global LAST_RESULT
    x = np.asarray(x)
    k_cache = np.asarray(k_cache)
    v_cache = np.asarray(v_cache)
    W_in = np.asarray(W_in, dtype=np.float32)
    b_in = np.asarray(b_in, dtype=np.float32)
    W_out = np.asarray(W_out, dtype=np.float32)
    b_out = np.asarray(b_out, dtype=np.float32)
    pos = np.asarray(input_pos).astype(np.int64)

    order = sorted(range(B), key=lambda b: -int(pos[b]))
    Ls = tuple(int(pos[b]) for b in order)
    nts = [(l + 127) // 128 for l in Ls]
    groups = make_groups(nts)

    key = (Ls, tuple(order))
    if key not in _build_cache:
        _build_cache[key] = _build(Ls, tuple(order))
    nc = _build_cache[key]

    x2 = np.ascontiguousarray(x.reshape(B, E)).astype(NPBF)
    kc_bf = k_cache.astype(NPBF)
    vc_bf = v_cache.astype(NPBF)

    in_maps = []
    for i in range(N_CORES):
        c0 = i * CLOC

        def pack_win(cols):
            w = np.ascontiguousarray(
                cols.reshape(ET, 128, CLOC).transpose(1, 0, 2).reshape(128, -1)
            ).astype(NPBF)
            return w

        winq_i = pack_win(W_in[:, c0 : c0 + CLOC]).reshape(128, ET, CLOC)
        wink_i = pack_win(W_in[:, E + c0 : E + c0 + CLOC]).reshape(128, ET, CLOC)
        winkq_i = np.ascontiguousarray(
            np.stack([winq_i, wink_i], axis=2).reshape(128, -1)
        )
        winv_i = pack_win(W_in[:, 2 * E + c0 : 2 * E + c0 + CLOC])
        bin_i = np.concatenate(
            [
                b_in[c0 : c0 + CLOC],
                b_in[E + c0 : E + c0 + CLOC],
                b_in[2 * E + c0 : 2 * E + c0 + CLOC],
            ]
        )[None, :].astype(NPBF)
        wout_i = np.ascontiguousarray(
            W_out[c0 : c0 + CLOC, :].reshape(H_LOC, 128, E)
            .transpose(1, 0, 2)
            .reshape(128, -1)
        ).astype(NPBF)
        h0 = i * H_LOC
        k_h = kc_bf[:, :, h0 : h0 + H_LOC, :]  # [B, S, 2, 128]
        v_h = vc_bf[:, :, h0 : h0 + H_LOC, :]
        k_blocks = []
        v_blocks = []
        for grp in groups:
            ksz = sum(2 * Ls[j] for j in grp)
            vsz = sum(nts[j] * VROW for j in grp)
            kg = np.zeros((128, ksz), dtype=NPBF)
            vg = np.zeros((128, vsz), dtype=NPBF)
            koff = 0
            voff = 0
            for j in grp:
                ob = order[j]
                L = Ls[j]
                nt = nts[j]
                if L > 1:
                    kb = np.zeros((128, H_LOC, L), dtype=NPBF)
                    kb[:, :, : L - 1] = k_h[ob, : L - 1].transpose(2, 1, 0)
                    kg[:, koff : koff + 2 * L] = kb.reshape(128, 2 * L)
                    vb = np.zeros((nt * 128, CLOC), dtype=NPBF)
                    vb[: L - 1] = v_h[ob, : L - 1].reshape(L - 1, CLOC)
                    vt = np.ones((nt, 128, VROW), dtype=NPBF)
                    vt[:, :, :CLOC] = vb.reshape(nt, 128, CLOC)
                    # [128 part, nt, VROW]
                    vg[:, voff : voff + nt * VROW] = (
                        vt.transpose(1, 0, 2).reshape(128, nt * VROW)
                    )
                else:
                    vt = np.ones((1, 128, VROW), dtype=NPBF)
                    vt[:, :, :CLOC] = 0
                    vg[:, voff : voff + VROW] = vt.reshape(128, VROW)
                koff += 2 * L
                voff += nt * VROW
            k_blocks.append(kg.ravel())
            v_blocks.append(vg.ravel())
        kc_i = np.concatenate(k_blocks)
        vc_i = np.concatenate(v_blocks)
        in_maps.append(
            {
                "x": x2,
                "winkq": winkq_i,
                "winv": winv_i,
                "bin": bin_i,
                "wout": wout_i,
                "kc": kc_i,
                "vc": vc_i,
            }
        )

    res = run_bass_kernel_spmd(nc, in_maps, core_ids=list(range(N_CORES)))
    LAST_RESULT = res
    acc = np.zeros((B, E), dtype=np.float64)
    for r in res.results:
        acc += r["out"].astype(np.float64)
    acc += b_out.astype(np.float64)
    out = np.zeros((B, E), dtype=np.float32)
    out[np.array(order)] = acc.astype(np.float32)
    return out.reshape(B, 1, E)


# revision 61
# speedup vs baseline: 1.1431x; 1.1431x over previous
"""Decode-step KV-cache attention kernel for 8 Trainium2 NeuronCores.

Tensor-parallel over heads (2 heads per core, all 32 batch rows on every
core); per-core differences live in host-sliced inputs.  All bulk data is
bf16 (tolerance is 2e-2; bf16 keeps norm-relative error ~4e-3 — fp8
anywhere in the attention path measures 2.4e-2+ because the attention
output is itself a noise-like average, so per-element quantization error
maps ~1:1 to relative output error).

Queue plan (the previous revision serialized the first K group behind a
3.1 MB W_in load on the same HWDGE queue and stalled every engine for
32 us):
  - scalar (HWDGE q10): K group tiles ONLY — first kt DMA issues at t=0.
  - gpsimd (SWDGE q0):  V group tiles ONLY.
  - sync   (HWDGE q1):  x, W_in (split q/k/v so the q columns land
    first and unblock the first scores), b_in, per-row v_new inserts,
    W_out (emitted late, it is only needed by phase 3), output stores.

Per-core pipeline (rows host-sorted by sequence length, descending;
adjacent small rows batched into shared 16 KB/partition DMA groups):
  1. QKV projection: x^T tiles (PE transposes), then q matmuls (winq),
     k_new matmuls (wink), v_new matmul (winv) — in that order so the
     first scores only wait on the 0.78 MB winq slice.
  2. Scores, per (row, head, tile): one matmul with the host-TRANSPOSED
     K tile [d, tokens] stationary and the q column moving -> scores
     [tokens, 1] in PSUM.  k_new/v_new are folded into the tiles at
     position L-1 beforehand.
  3. Exp per row over the packed score tiles (exact lengths, exact
     denominators).
  4. PV, per (row, tile): ONE matmul with the 2 probability columns as
     weights and the V tile [tokens, 257] streaming — column 256 of
     every V row-tile is hardwired to 1.0 host-side, so the softmax
     denominator accumulates in PSUM column 256 of the same bank and
     the separate denominator matmul (and its weight load) is gone.
  5. Per row: reciprocal, normalize out of PSUM, PE-transpose into
     [d, rowhead] layout.
  6. Out-project with bf16 W_out, PSUM->SBUF->HBM in 512-col chunks;
     host sums the 8 per-core partials and adds b_out.
"""

import math
import sys

import numpy as np
import ml_dtypes

sys.path.insert(0, "/opt/trn_rl_repo")

import concourse.bass as bass  # noqa: E402
import concourse.tile as tile  # noqa: E402
from concourse import bacc, mybir  # noqa: E402
from concourse.bass_utils import run_bass_kernel_spmd  # noqa: E402
from concourse.masks import make_identity  # noqa: E402

B, S_MAX, H, D = 32, 2048, 16, 128
E = H * D  # 2048
N_CORES = 8
H_LOC = H // N_CORES  # 2 heads per core
CLOC = H_LOC * D  # 256
VROW = CLOC + 1  # V tile row: 256 d-columns + 1 ones-column (denominator)
ET = E // 128  # 16 contraction tiles for the in-projection

F32 = mybir.dt.float32
BF16 = mybir.dt.bfloat16
NPBF = ml_dtypes.bfloat16
EXP = mybir.ActivationFunctionType.Exp

_build_cache: dict = {}
LAST_RESULT = None  # last BassKernelResults, for test harness introspection


def make_groups(nts):
    """Group adjacent rows so several small rows share one DMA + SBUF tile.
    Group size n at max-tile-count m keeps n*m <= 32 (16 KB/partition); a
    strict pad budget keeps the padding bytes negligible."""
    groups = []
    j = 0
    while j < B:
        m = nts[j]
        n = 1
        pad = 0
        while j + n < B and n < 16:
            m2 = max(m, nts[j + n])
            pad2 = pad + (m2 - m) * n + (m2 - nts[j + n])
            # cap at 31 tiles: keeps every DMA partition line <= 16 KB so no
            # transfer straddles the SDMA packet boundary
            if (n + 1) * m2 > 31 or pad2 > 2:
                break
            m, pad = m2, pad2
            n += 1
        groups.append(list(range(j, j + n)))
        j += n
    return groups


def _build(Ls: tuple, obs: tuple) -> bass.Bass:
    """Per-core Bass program. Ls = sorted (descending) seq lengths;
    obs[j] = original batch index of sorted row j."""
    nts = [(l + 127) // 128 for l in Ls]
    groups = make_groups(nts)
    g_of = {}
    r_of = {}
    k_rof = {}  # row's K base offset within its group tile (exact lengths)
    v_rof = {}  # row's V base offset within its group tile (exact tiles)
    for gi, grp in enumerate(groups):
        koff = 0
        voff = 0
        for r, j in enumerate(grp):
            g_of[j], r_of[j] = gi, r
            k_rof[j] = koff
            v_rof[j] = voff
            koff += 2 * Ls[j]
            voff += nts[j] * VROW
    k_goff = []  # element offsets of each group's K/V block in packed bufs
    v_goff = []
    k_gsz = []
    v_gsz = []
    ko = vo = 0
    for gi, grp in enumerate(groups):
        k_goff.append(ko)
        v_goff.append(vo)
        k_gsz.append(sum(2 * Ls[j] for j in grp))
        v_gsz.append(sum(nts[j] * VROW for j in grp))
        ko += 128 * k_gsz[gi]
        vo += 128 * v_gsz[gi]

    nc = bacc.Bacc("TRN2")
    x_d = nc.dram_tensor("x", [B, E], BF16, kind="ExternalInput")
    winkq_d = nc.dram_tensor(
        "winkq", [128, ET * 2 * CLOC], BF16, kind="ExternalInput"
    )
    winv_d = nc.dram_tensor("winv", [128, ET * CLOC], BF16, kind="ExternalInput")
    bin_d = nc.dram_tensor("bin", [1, 3 * CLOC], BF16, kind="ExternalInput")
    wout_d = nc.dram_tensor("wout", [128, H_LOC * E], BF16, kind="ExternalInput")
    kc_d = nc.dram_tensor("kc", [ko], BF16, kind="ExternalInput")
    vc_d = nc.dram_tensor("vc", [vo], BF16, kind="ExternalInput")
    out_d = nc.dram_tensor("out", [B, E], F32, kind="ExternalOutput")

    kc_base = kc_d[:]
    vc_base = vc_d[:]
    inv_sqrt_d = 1.0 / math.sqrt(D)

    with tile.TileContext(nc) as tc:
        with tc.tile_pool(name="const", bufs=1) as const:
            I64 = const.tile([64, 64], BF16)
            make_identity(nc, I64)
            I32 = I64[0:32, 0:32]
            ones_1x32 = const.tile([1, 32], BF16)
            nc.vector.memset(ones_1x32, 1.0)

            # All phase-1 weights go FIRST on the scalar queue — ahead of the
            # K groups in its FIFO — while the V stream is WAR-held behind
            # qT: during the first ~10 us the weight blob owns the SDMA
            # engines and lands at full rate.
            x_sb = const.tile([B, E], BF16)
            nc.scalar.dma_start(out=x_sb, in_=x_d[:])
            bin_sb = const.tile([1, 3 * CLOC], BF16)
            nc.scalar.dma_start(out=bin_sb, in_=bin_d[:])
            # k_new and q weight slices ride ONE transfer (one semaphore:
            # their consumers cannot be poisoned by each other's traffic).
            # It precedes the K groups in the scalar FIFO while the V stream
            # is WAR-held behind qT, so the weights land at full rate first.
            winkq_sb = const.tile([128, ET, 2, CLOC], BF16)
            nc.scalar.dma_start(out=winkq_sb, in_=winkq_d[:])
            winv_sb = const.tile([128, ET, CLOC], BF16)
            nc.scalar.dma_start(out=winv_sb, in_=winv_d[:])
            wout_sb = const.tile([128, H_LOC, E], BF16)
            # preload the exp table during the initial DMA window
            dummy_sb = const.tile([1, 2], F32)
            nc.scalar.activation(
                dummy_sb[0:1, 0:1], ones_1x32[0:1, 0:1], EXP, scale=1.0
            )

            xT_sb = const.tile([128, ET, B], BF16)
            v_new_sb = const.tile([B, CLOC], BF16)
            qT_sb = const.tile([128, H_LOC, B], BF16)
            k_newT_sb = const.tile([128, H_LOC, B], BF16)
            aT_sb = const.tile([128, H_LOC, 2 * B], BF16)
            out_sb = const.tile([B, E], F32)

            # ---------------- phase 1: fused QKV projection ----------------
            # q and k_new are produced directly in [d, row] orientation by
            # using the W_in columns as the output partition dim.  k_new
            # first (the per-row cache inserts consume it), then q, then
            # v_new from winv whose DMA is WAR-anchored behind qT so it can
            # never delay the score-critical winkq transfer.
            with tc.tile_pool(name="ph1ps", bufs=2, space="PSUM") as ph1ps:
                with tc.tile_pool(name="qkvps", bufs=1, space="PSUM") as qkvps:
                    for t in range(ET):
                        xt_ps = ph1ps.tile([128, B], BF16)
                        nc.tensor.transpose(
                            xt_ps, x_sb[:, t * 128 : (t + 1) * 128], I32
                        )
                        nc.vector.tensor_copy(xT_sb[:, t, :], xt_ps)
                    for sel, bbase, dst in (
                        (1, CLOC, k_newT_sb),
                        (0, 0, qT_sb),
                    ):
                        for h in range(H_LOC):
                            ps = qkvps.tile([128, B], F32, tag=f"qk{sel}{h}")
                            nc.tensor.matmul(
                                ps,
                                bin_sb[:, bbase + h * D : bbase + (h + 1) * D],
                                ones_1x32,
                                start=True,
                                stop=False,
                            )
                            for t in range(ET):
                                nc.tensor.matmul(
                                    ps,
                                    winkq_sb[
                                        :, t, sel, h * D : (h + 1) * D
                                    ],
                                    xT_sb[:, t, :],
                                    start=False,
                                    stop=(t == ET - 1),
                                )
                            nc.vector.tensor_copy(dst[:, h, :], ps)
                    v_ps = qkvps.tile([B, CLOC], F32, tag="v")
                    nc.tensor.matmul(
                        v_ps,
                        ones_1x32,
                        bin_sb[:, 2 * CLOC : 3 * CLOC],
                        start=True,
                        stop=False,
                    )
                    for t in range(ET):
                        nc.tensor.matmul(
                            v_ps,
                            xT_sb[:, t, :],
                            winv_sb[:, t, :],
                            start=False,
                            stop=(t == ET - 1),
                        )
                    nc.vector.tensor_copy(v_new_sb, v_ps)

            # ---------------- phase 2: scores -> exp -> PV ------------------
            with tc.tile_pool(name="scps", bufs=3, space="PSUM") as scps, \
                 tc.tile_pool(name="avps", bufs=2, space="PSUM") as avps, \
                 tc.tile_pool(name="tps", bufs=2, space="PSUM") as tps, \
                 tc.tile_pool(name="ktp", bufs=5) as ktp, \
                 tc.tile_pool(name="vtp", bufs=4) as vtp, \
                 tc.tile_pool(name="prp", bufs=4) as prp, \
                 tc.tile_pool(name="arp", bufs=3) as arp:
                kts = [None] * len(groups)
                vts = [None] * len(groups)
                prs = [None] * B
                GELEMS = 8192  # 16 KB/partition K group tiles
                VGELEMS = 32 * VROW  # V group tiles incl. ones-columns

                # The smallest rows (last 3 groups) are preloaded into
                # dedicated tiles on the otherwise-idle sync queue at t=0 and
                # processed mid-stream: the post-DMA drain then ends with the
                # 4th-last group instead of rows that only arrive at the very
                # end of the stream.
                S_groups = set()  # tail preloading measured slower; disabled
                for gi in sorted(S_groups):
                    kt = const.tile([128, k_gsz[gi]], BF16, tag=f"tk{gi}")
                    kts[gi] = kt
                    nc.sync.dma_start(
                        out=kt,
                        in_=bass.AP(
                            tensor=kc_base.tensor,
                            offset=k_goff[gi],
                            ap=[[k_gsz[gi], 128], [1, k_gsz[gi]]],
                        ),
                    )
                    vt = const.tile([128, v_gsz[gi]], BF16, tag=f"tv{gi}")
                    vts[gi] = vt
                    nc.sync.dma_start(
                        out=vt,
                        in_=bass.AP(
                            tensor=vc_base.tensor,
                            offset=v_goff[gi],
                            ap=[[v_gsz[gi], 128], [1, v_gsz[gi]]],
                        ),
                    )

                def anchor(dst_ap, src_ap):
                    """WAR anchor: a 1-element DVE write into the DMA target
                    region, reading a value produced by the stage the DMA
                    must trail — the transfer can then neither be hoisted
                    nor issued before that stage retires."""
                    nc.vector.tensor_copy(dst_ap, src_ap)

                emitted = []

                def anchor_src():
                    # prp rotates every 4 tiles, so this 6-back read aliases
                    # to the 2-back tile's buffer — an effective ~2-row trail,
                    # which measured best
                    if emitted:
                        return prs[emitted[-6] if len(emitted) >= 6
                                   else emitted[0]]
                    return None

                def emit_inserts(gi):
                    # cross-partition v_new inserts must go through DMA;
                    # hoisted to group level so their conservative
                    # cross-engine sem thresholds stay small
                    vt = vts[gi]
                    for jj in groups[gi]:
                        cj = Ls[jj] - 1
                        pn, tn = cj % 128, cj // 128
                        vjo = v_rof[jj]
                        oj = obs[jj]
                        nc.sync.dma_start(
                            out=vt[
                                pn : pn + 1,
                                vjo + tn * VROW : vjo + tn * VROW + CLOC,
                            ],
                            in_=v_new_sb[oj : oj + 1, :],
                        )

                def emit_scores(j):
                    L = Ls[j]
                    nt = nts[j]
                    gi, r = g_of[j], r_of[j]
                    ksz = k_gsz[gi]
                    if r == 0 and gi in S_groups:
                        emit_inserts(gi)
                    elif r == 0:
                        kt = ktp.tile([128, GELEMS], BF16)
                        kts[gi] = kt
                        src = anchor_src() if gi >= 1 else None
                        if src is not None:
                            anchor(kt[0:1, 0:1], src[0:1, 0, 0:1])
                        nc.scalar.dma_start(
                            out=kt[:, 0:ksz],
                            in_=bass.AP(
                                tensor=kc_base.tensor,
                                offset=k_goff[gi],
                                ap=[[ksz, 128], [1, ksz]],
                            ),
                        )
                        vt = vtp.tile([128, VGELEMS], BF16)
                        vts[gi] = vt
                        vsz = v_gsz[gi]
                        if gi < 2:
                            # V is not needed until the first PV (~LAG rows
                            # in); holding its head start keeps the startup
                            # window clear for weights + first K
                            anchor(vt[0:1, 0:1], qT_sb[0:1, 0, 0:1])
                        else:
                            src = anchor_src()
                            if src is not None:
                                anchor(vt[0:1, 0:1], src[0:1, 0, 0:1])
                        nc.gpsimd.dma_start(
                            out=vt[:, 0:vsz],
                            in_=bass.AP(
                                tensor=vc_base.tensor,
                                offset=v_goff[gi],
                                ap=[[vsz, 128], [1, vsz]],
                            ),
                        )
                        emit_inserts(gi)
                    kt = kts[gi]
                    ko = k_rof[j]  # this row's K base within the tile
                    # fold the new token in at column L-1
                    col = L - 1
                    ob = obs[j]
                    for h in range(H_LOC):
                        nc.vector.tensor_copy(
                            kt[:, ko + h * L + col : ko + h * L + col + 1],
                            k_newT_sb[:, h, ob : ob + 1],
                        )
                    sc = scps.tile([128, H_LOC, ET], F32)
                    pr = prp.tile([128, H_LOC, ET], BF16)
                    prs[j] = pr
                    rem = L - (nt - 1) * 128  # tokens in the last tile
                    for h in range(H_LOC):
                        for t in range(nt):
                            o = ko + h * L + t * 128
                            nc2 = 128 if t < nt - 1 else rem
                            nc.tensor.matmul(
                                sc[0:nc2, h, t : t + 1],
                                kt[:, o : o + nc2],
                                qT_sb[:, h, ob : ob + 1],
                                start=True,
                                stop=True,
                                skip_group_check=True,
                            )
                    if nt > 1:
                        nc.scalar.activation(
                            pr[:, :, 0 : nt - 1],
                            sc[:, :, 0 : nt - 1],
                            EXP,
                            scale=inv_sqrt_d,
                        )
                    nc.scalar.activation(
                        pr[0:rem, :, nt - 1 : nt],
                        sc[0:rem, :, nt - 1 : nt],
                        EXP,
                        scale=inv_sqrt_d,
                    )

                def emit_pv(j):
                    nt = nts[j]
                    gi, r = g_of[j], r_of[j]
                    vt = vts[gi]
                    vo = v_rof[j]
                    pr = prs[j]
                    # av columns 0:256 accumulate P@V; column 256 accumulates
                    # the softmax denominator via the ones-column of each V
                    # row-tile (same PSUM bank, same accumulation stream).
                    av = avps.tile([H_LOC, VROW], F32)
                    rem = Ls[j] - (nt - 1) * 128
                    for t in range(nt):
                        nc2 = 128 if t < nt - 1 else rem
                        nc.tensor.matmul(
                            av,
                            pr[0:nc2, :, t],
                            vt[0:nc2, vo + t * VROW : vo + (t + 1) * VROW],
                            start=(t == 0),
                            stop=(t == nt - 1),
                            skip_group_check=True,
                        )
                    ar = arp.tile([H_LOC, CLOC], BF16)
                    den2 = arp.tile([H_LOC, 2], F32, tag="dn")
                    nc.vector.reciprocal(den2[:, 1:2], av[:, CLOC : CLOC + 1])
                    nc.vector.tensor_scalar_mul(ar, av[:, 0:CLOC], den2[:, 1:2])
                    for h in range(H_LOC):
                        at_ps = tps.tile([128, H_LOC], BF16)
                        nc.tensor.transpose(
                            at_ps, ar[:, h * D : (h + 1) * D], I64[0:2, 0:2]
                        )
                        nc.vector.tensor_copy(
                            aT_sb[:, h, 2 * j : 2 * j + 2], at_ps
                        )

                LAG = 3
                WOUT_AT = 24  # start the 1 MB wout load ~75% through
                tail_rows = [
                    jj for gi in sorted(S_groups) for jj in groups[gi]
                ]
                head_rows = [
                    jj for jj in range(B) if g_of[jj] not in S_groups
                ]
                seq = head_rows[:6] + tail_rows + head_rows[6:]
                for i, j in enumerate(seq):
                    emit_scores(j)
                    emitted.append(j)
                    if i >= LAG:
                        emit_pv(seq[i - LAG])
                    if i == WOUT_AT:
                        # WAR anchor: the copy below reads the aT slot of the
                        # row finished LAG steps ago, so the wout DMA cannot
                        # be hoisted ahead of it — it streams only once ~75%
                        # of the pipeline has drained.
                        pv = seq[i - LAG]
                        nc.vector.tensor_copy(
                            wout_sb[0:1, 0, 0:2],
                            aT_sb[0:1, 0, 2 * pv : 2 * pv + 2],
                        )
                        nc.sync.dma_start(out=wout_sb, in_=wout_d[:])
                for i in range(B - LAG, B):
                    emit_pv(seq[i])

            # ---------------- phase 3: out-projection -----------------------
            with tc.tile_pool(name="outps", bufs=1, space="PSUM") as outps:
                    out_ps = outps.tile([B, E], F32)
                    # all matmuls first (independent PSUM banks accumulate
                    # back-to-back, keeping the PE ramped), copies + stores
                    # drain behind them
                    for j4 in range(4):
                        for h in range(H_LOC):
                            base = aT_sb[:, h, :]
                            lhsT = bass.AP(
                                tensor=base.tensor,
                                offset=base.offset + h,
                                ap=[base.ap[0], [2, B]],
                            )
                            nc.tensor.matmul(
                                out_ps[:, j4 * 512 : (j4 + 1) * 512],
                                lhsT,
                                wout_sb[:, h, j4 * 512 : (j4 + 1) * 512],
                                start=(h == 0),
                                stop=(h == H_LOC - 1),
                                skip_group_check=True,
                            )
                    for j4 in range(4):
                        s = slice(j4 * 512, (j4 + 1) * 512)
                        nc.vector.tensor_copy(out_sb[:, s], out_ps[:, s])
                        nc.sync.dma_start(out=out_d[:, s], in_=out_sb[:, s])
    nc.compile()
    return nc


def kernel(x, k_cache, v_cache, W_in, b_in, W_out, b_out, input_pos):
    global LAST_RESULT
    x = np.asarray(x)
    k_cache = np.asarray(k_cache)
    v_cache = np.asarray(v_cache)
    W_in = np.asarray(W_in, dtype=np.float32)
    b_in = np.asarray(b_in, dtype=np.float32)
    W_out = np.asarray(W_out, dtype=np.float32)
    b_out = np.asarray(b_out, dtype=np.float32)
    pos = np.asarray(input_pos).astype(np.int64)

    order = sorted(range(B), key=lambda b: -int(pos[b]))
    Ls = tuple(int(pos[b]) for b in order)
    nts = [(l + 127) // 128 for l in Ls]
    groups = make_groups(nts)

    key = (Ls, tuple(order))
    if key not in _build_cache:
        _build_cache[key] = _build(Ls, tuple(order))
    nc = _build_cache[key]

    x2 = np.ascontiguousarray(x.reshape(B, E)).astype(NPBF)
    kc_bf = k_cache.astype(NPBF)
    vc_bf = v_cache.astype(NPBF)

    in_maps = []
    for i in range(N_CORES):
        c0 = i * CLOC

        def pack_win(cols):
            w = np.ascontiguousarray(
                cols.reshape(ET, 128, CLOC).transpose(1, 0, 2).reshape(128, -1)
            ).astype(NPBF)
            return w

        winq_i = pack_win(W_in[:, c0 : c0 + CLOC]).reshape(128, ET, CLOC)
        wink_i = pack_win(W_in[:, E + c0 : E + c0 + CLOC]).reshape(128, ET, CLOC)
        winkq_i = np.ascontiguousarray(
            np.stack([winq_i, wink_i], axis=2).reshape(128, -1)
        )
        winv_i = pack_win(W_in[:, 2 * E + c0 : 2 * E + c0 + CLOC])
        bin_i = np.concatenate(
            [
                b_in[c0 : c0 + CLOC],
                b_in[E + c0 : E + c0 + CLOC],
                b_in[2 * E + c0 : 2 * E + c0 + CLOC],
            ]
        )[None, :].astype(NPBF)
        wout_i = np.ascontiguousarray(
            W_out[c0 : c0 + CLOC, :].reshape(H_LOC, 128, E)
            .transpose(1, 0, 2)
            .reshape(128, -1)
        ).astype(NPBF)
        h0 = i * H_LOC
        k_h = kc_bf[:, :, h0 : h0 + H_LOC, :]  # [B, S, 2, 128]
        v_h = vc_bf[:, :, h0 : h0 + H_LOC, :]
        k_blocks = []
        v_blocks = []
        for grp in groups:
            ksz = sum(2 * Ls[j] for j in grp)
            vsz = sum(nts[j] * VROW for j in grp)
            kg = np.zeros((128, ksz), dtype=NPBF)
            vg = np.zeros((128, vsz), dtype=NPBF)
            koff = 0
            voff = 0
            for j in grp:
                ob = order[j]
                L = Ls[j]
                nt = nts[j]
                if L > 1:
                    kb = np.zeros((128, H_LOC, L), dtype=NPBF)
                    kb[:, :, : L - 1] = k_h[ob, : L - 1].transpose(2, 1, 0)
                    kg[:, koff : koff + 2 * L] = kb.reshape(128, 2 * L)
                    vb = np.zeros((nt * 128, CLOC), dtype=NPBF)
                    vb[: L - 1] = v_h[ob, : L - 1].reshape(L - 1, CLOC)
                    vt = np.ones((nt, 128, VROW), dtype=NPBF)
                    vt[:, :, :CLOC] = vb.reshape(nt, 128, CLOC)
                    # [128 part, nt, VROW]
                    vg[:, voff : voff + nt * VROW] = (
                        vt.transpose(1, 0, 2).reshape(128, nt * VROW)
                    )
                else:
                    vt = np.ones((1, 128, VROW), dtype=NPBF)
                    vt[:, :, :CLOC] = 0
                    vg[:, voff : voff + VROW] = vt.reshape(128, VROW)
                koff += 2 * L
                voff += nt * VROW
            k_blocks.append(kg.ravel())
            v_blocks.append(vg.ravel())
        kc_i = np.concatenate(k_blocks)
        vc_i = np.concatenate(v_blocks)
        in_maps.append(
            {
                "x": x2,
                "winkq": winkq_i,
                "winv": winv_i,
                "bin": bin_i,
                "wout": wout_i,
                "kc": kc_i,
                "vc": vc_i,
            }
        )

    res = run_bass_kernel_spmd(nc, in_maps, core_ids=list(range(N_CORES)))
    LAST_RESULT = res
    acc = np.zeros((B, E), dtype=np.float64)
    for r in res.results:
        acc += r["out"].astype(np.float64)
    acc += b_out.astype(np.float64)
    out = np.zeros((B, E), dtype=np.float32)
    out[np.array(order)] = acc.astype(np.float32)
    return out.reshape(B, 1, E)
